# revision 1
# baseline (speedup 1.0000x reference)
"""Trainium2 Bass kernel for nn_Attention_54520314855575.

GQA attention with raw row-major reshapes (faithful to reference). The raw
reshapes scramble heads/tokens such that each query head's 64 output rows are
disjoint across heads -> shard 8 ways (2 batches x 4 head-groups) with zero
collectives. Per core: 8 query heads (hq%8 in {2r, 2r+1}), K/V heads {2r,2r+1}.

Compute: bf16 matmuls, f32 PSUM/softmax. All projections computed transposed
(channels on partitions) so biases are per-partition; V computed natural and
shuffled on-chip into (t, d) layout.

t-axis permutation: within each 128-row t-tile, partition p holds t-offset
8*(p%16) + p//16 (so the V shuffle writes 16 contiguous partitions per
c-chunk). The scores lhsT (KT) is stored in the same order and the diagonal
mask rows are permuted on host, so the contraction stays consistent.

Weights are host-pretiled so every weight-slab DMA is one contiguous block.
"""
import sys, os

for _p in ("/opt/trn_rl_repo",):
    if _p not in sys.path:
        sys.path.append(_p)

import numpy as np
import ml_dtypes

import concourse.bass as bass
import concourse.tile as tile
from concourse import bacc, mybir
from concourse.bass_utils import run_bass_kernel_spmd

BF16 = mybir.dt.bfloat16
F32 = mybir.dt.float32

H = 4096; HQ = 32; HK = 8; HV = 8; DQ = 128; DV = 512; S = 2048; B = 2
NEG = -1.0e30

_CACHE = {}


def build(causal: bool):
    nc = bacc.Bacc(None, target_bir_lowering=False, debug=False)

    xq_d = nc.declare_dram_parameter("xq", [128, 32, 512], BF16, isOutput=False)
    xkv_d = nc.declare_dram_parameter("xkv", [128, 32, 512], BF16, isOutput=False)
    wq_d = nc.declare_dram_parameter("wq", [8, 32, 128, 512], BF16, isOutput=False)
    bq_d = nc.declare_dram_parameter("bq", [128, 32], F32, isOutput=False)
    wk_d = nc.declare_dram_parameter("wk", [2, 32, 128, 512], BF16, isOutput=False)
    bk_d = nc.declare_dram_parameter("bk", [128, 8], F32, isOutput=False)
    wv_d = nc.declare_dram_parameter("wv", [8, 32, 128, 512], BF16, isOutput=False)
    bV_d = nc.declare_dram_parameter("bV", [128, 512], BF16, isOutput=False)
    w0_d = nc.declare_dram_parameter("w0", [4, 128, 128, 1024], BF16, isOutput=False)
    bias_plane_d = nc.declare_dram_parameter("bias_plane", [4096, 512], F32, isOutput=False)
    mask_diag_d = nc.declare_dram_parameter("mask_diag", [128, 128], F32, isOutput=False)
    if not causal:
        maskT_d = nc.declare_dram_parameter("maskT", [16, 128, 4, 512], BF16, isOutput=False)
    outT_d = nc.declare_dram_parameter("outT", [4096, 512], F32, isOutput=True)

    with tile.TileContext(nc) as tc:
        with tc.tile_pool(name="const", bufs=1) as constp, \
             tc.tile_pool(name="qkv", bufs=1) as qkvp, \
             tc.tile_pool(name="dram", bufs=1, space="DRAM") as dramp:

            mask_sb = constp.tile([128, 128], F32)
            nc.sync.dma_start(mask_sb[:], mask_diag_d[:])
            bV_sb = constp.tile([128, 512], BF16)
            nc.sync.dma_start(bV_sb[:], bV_d[:])
            bq_sb = constp.tile([128, 32], F32)
            nc.sync.dma_start(bq_sb[:], bq_d[:])
            bk_sb = constp.tile([128, 8], F32)
            nc.sync.dma_start(bk_sb[:], bk_d[:])
            ones_f = constp.tile([128, 1], F32)
            nc.vector.memset(ones_f[:], 1.0)
            ones_r = constp.tile([128, 1], mybir.dt.float32r)
            nc.vector.tensor_copy(ones_r[:], ones_f[:])

            QT = qkvp.tile([128, 8, 2048], BF16)   # [d, head hloc, q]
            KT = qkvp.tile([128, 2, 2048], BF16)   # [d, head j0, tperm]
            Vsh = qkvp.tile([128, 2, 16, 512], BF16)  # [pnew, head j0, ttile, d]
            ctx_dram = dramp.tile([32, 4, 128, 512], BF16)  # [sm, dd, dpart, s']
            vf_dram = dramp.tile([512, 4096], BF16)         # natural (tokloc, c)

            # ---------------- Phase 1: projections ----------------
            with tc.tile_pool(name="xres", bufs=1) as xp, \
                 tc.tile_pool(name="wstr", bufs=4) as wp, \
                 tc.tile_pool(name="vtmp", bufs=3) as vtp, \
                 tc.tile_pool(name="pps", bufs=8, space="PSUM") as pps:

                xq_sb = xp.tile([128, 32, 512], BF16)
                nc.sync.dma_start(xq_sb[:], xq_d[:])
                xkv_sb = xp.tile([128, 32, 512], BF16)
                nc.sync.dma_start(xkv_sb[:], xkv_d[:])

                # QT: lhsT = wq block (h128, c128), rhs = xq (h128, tok512)
                for cg in range(8):
                    acc = [pps.tile([128, 512], F32, tag="pj", name=f"pj{_}") for _ in range(4)]
                    for h in range(32):
                        wsl = wp.tile([128, 512], BF16, tag="w")
                        nc.sync.dma_start(wsl[:], wq_d[cg, h, :, :])
                        for i in range(4):
                            nc.tensor.matmul(acc[i][:], wsl[:, 128*i:128*i+128],
                                             xq_sb[:, h, :], start=(h == 0), stop=(h == 31))
                    for i in range(4):
                        ct = 4*cg + i  # == sm
                        # QT[p, hd, q=32u+sm] <- acc[p, tok=128k+64j0+u], hd=2k+j0
                        out = QT[:].rearrange("p hd (u sm) -> p hd u sm", sm=32)[:, :, :, ct]
                        nc.vector.tensor_scalar_add(
                            out, acc[i][:].rearrange("p (hd u) -> p hd u", hd=8),
                            bq_sb[:, ct:ct+1])

                # KT: new within-tile t order: free = 128*i4 + 16*cc + m
                for cg in range(2):
                    acc = [pps.tile([128, 512], F32, tag="pj", name=f"pj{_}") for _ in range(4)]
                    for h in range(32):
                        wsl = wp.tile([128, 512], BF16, tag="w")
                        nc.sync.dma_start(wsl[:], wk_d[cg, h, :, :])
                        for i in range(4):
                            nc.tensor.matmul(acc[i][:], wsl[:, 128*i:128*i+128],
                                             xkv_sb[:, h, :], start=(h == 0), stop=(h == 31))
                    for i in range(4):
                        ct = 4*cg + i  # == cc
                        # KT[p, hd, 128*i4 + 16*cc + m] <- acc[p, tok=256*hd+16*i4+m]
                        out = KT[:].rearrange("p hd (i4 cc m) -> p hd i4 cc m",
                                              cc=8, m=16)[:, :, :, ct, :]
                        nc.vector.tensor_scalar_add(
                            out, acc[i][:].rearrange("p (hd i4 m) -> p hd i4 m", hd=2, i4=16),
                            bk_sb[:, ct:ct+1])

                # V natural: lhsT = xkv block (h128, tok128), rhs = wv (h128, c512)
                for ccg in range(8):
                    acc = [pps.tile([128, 512], F32, tag="pj", name=f"pj{_}") for _ in range(4)]
                    for h in range(32):
                        wsl = wp.tile([128, 512], BF16, tag="w")
                        nc.sync.dma_start(wsl[:], wv_d[ccg, h, :, :])
                        for tt in range(4):
                            nc.tensor.matmul(acc[tt][:], xkv_sb[:, h, 128*tt:128*tt+128],
                                             wsl[:], start=(h == 0), stop=(h == 31))
                    for tt in range(4):
                        vnat = vtp.tile([128, 512], BF16, tag="vn")
                        nc.vector.tensor_copy(vnat[:], acc[tt][:])
                        nc.sync.dma_start(
                            vf_dram[128*tt:128*tt+128, 512*ccg:512*ccg+512], vnat[:])

                # gather V (t, d) tiles from DRAM: pnew = 16*cc + m holds
                # t = 128*i4 + 8*m + cc, i.e. Vf[256*j0 + 16*i4 + m, cc*512 + d]
                for j0 in range(2):
                    for i4 in range(16):
                        for cc in range(8):
                            r0_ = 256*j0 + 16*i4
                            nc.sync.dma_start(
                                Vsh[16*cc:16*cc+16, j0, i4, :],
                                vf_dram[r0_:r0_+16, 512*cc:512*cc+512])

                # V bias: V[pnew, d] += bV[pnew, d] (host permuted)
                for j0 in range(2):
                    for i4 in range(16):
                        nc.vector.tensor_add(Vsh[:, j0, i4, :], Vsh[:, j0, i4, :], bV_sb[:])

            # ---------------- Phase 2: attention ----------------
            with tc.tile_pool(name="esb", bufs=2) as ep, \
                 tc.tile_pool(name="nrm", bufs=2) as np_, \
                 tc.tile_pool(name="cev", bufs=4) as cevp, \
                 tc.tile_pool(name="mstr", bufs=4) as mp, \
                 tc.tile_pool(name="aps", bufs=1, space="PSUM") as aps:

                for hloc in range(8):
                    k, j0 = hloc // 2, hloc % 2
                    for c in range(4):
                        nt = 4*c + 4 if causal else 16
                        E = ep.tile([128, 16, 512], BF16, tag="E")
                        Esum = ep.tile([128, 512], mybir.dt.float32r, tag="Esum")
                        rs_ps = aps.tile([1, 512], F32, tag="rs")
                        pctx = [aps.tile([128, 512], F32, tag=f"ctx{dd}", name=f"ctx{dd}")
                                for dd in range(4)]
                        for i4 in range(nt):
                            sc_ps = aps.tile([128, 512], F32, tag="sc", bufs=2)
                            nc.tensor.matmul(
                                sc_ps[:], KT[:, j0, 128*i4:128*i4+128],
                                QT[:, hloc, 512*c:512*c+512], start=True, stop=True)
                            if causal:
                                if 4*c <= i4 < 4*c+4:
                                    q0 = 128*(i4 - 4*c)
                                    nc.vector.tensor_add(sc_ps[:, q0:q0+128],
                                                         sc_ps[:, q0:q0+128], mask_sb[:])
                                m0 = 128*(i4 - 4*c) if i4 > 4*c else 0
                            else:
                                msk = mp.tile([128, 512], BF16, tag="mk")
                                nc.sync.dma_start(msk[:], maskT_d[i4, :, c, :])
                                nc.vector.tensor_add(sc_ps[:], sc_ps[:], msk[:])
                                m0 = 0
                            if m0 > 0:
                                nc.vector.memset(E[:, i4, 0:m0], 0.0)
                            nc.scalar.activation(E[:, i4, m0:512], sc_ps[:, m0:512],
                                                 mybir.ActivationFunctionType.Exp)
                            if i4 == 0:
                                nc.vector.tensor_copy(Esum[:], E[:, 0, :])
                            else:
                                nc.vector.tensor_add(Esum[:], Esum[:], E[:, i4, :])
                            for dd in range(4):
                                nc.tensor.matmul(pctx[dd][:],
                                                 Vsh[:, j0, i4, 128*dd:128*dd+128],
                                                 E[:, i4, :],
                                                 start=(i4 == 0), stop=(i4 == nt-1))
                        nc.tensor.matmul(rs_ps[:], ones_r[:], Esum[:],
                                         start=True, stop=True)
                        # short normalize chain: recip on (1,512), then broadcast
                        rs_sb = np_.tile([1, 512], F32, tag="rssb")
                        nc.vector.tensor_copy(rs_sb[:], rs_ps[:])
                        rc1_sb = np_.tile([1, 512], F32, tag="rc1")
                        nc.vector.reciprocal(rc1_sb[:], rs_sb[:])
                        rc_sb = np_.tile([128, 512], F32, tag="rc")
                        nc.gpsimd.partition_broadcast(rc_sb[:], rc1_sb[:])
                        for dd in range(4):
                            # evict in sm-major order so the DRAM store is u-contiguous
                            cev = cevp.tile([128, 512], BF16, tag="cev")
                            perm = "p (u sm) -> p sm u"
                            nc.vector.tensor_mul(
                                cev[:],
                                pctx[dd][:].rearrange(perm, sm=32),
                                rc_sb[:].rearrange(perm, sm=32))
                            src = cev[:].rearrange("p (sm u) -> p sm u", u=16)
                            dst = ctx_dram[:].rearrange("sm dd dp s -> dp sm dd s")[
                                :, :, dd, 64*hloc+16*c:64*hloc+16*c+16]
                            nc.sync.dma_start(dst.opt(), src.opt())

            # ---------------- Phase 3: output projection ----------------
            with tc.tile_pool(name="w0str", bufs=4) as w0p, \
                 tc.tile_pool(name="rhsp", bufs=4) as rhp, \
                 tc.tile_pool(name="evo", bufs=3) as evp, \
                 tc.tile_pool(name="wps", bufs=1, space="PSUM") as wps:

                for og in range(4):
                    pout = [wps.tile([128, 512], F32, tag=f"o{o}", name=f"po{o}")
                            for o in range(8)]
                    for ft in range(128):
                        sm, dd = ft // 4, ft % 4
                        rhs = rhp.tile([128, 512], BF16, tag="rhs")
                        nc.sync.dma_start(rhs[:], ctx_dram[sm, dd, :, :])
                        wsl = w0p.tile([128, 1024], BF16, tag="w0")
                        nc.sync.dma_start(wsl[:], w0_d[og, ft, :, :])
                        for o in range(8):
                            nc.tensor.matmul(pout[o][:], wsl[:, 128*o:128*o+128], rhs[:],
                                             start=(ft == 0), stop=(ft == 127))
                    for o in range(8):
                        orow = 1024*og + 128*o
                        bsl = evp.tile([128, 512], F32, tag="bp")
                        nc.sync.dma_start(bsl[:], bias_plane_d[orow:orow+128, :])
                        res = evp.tile([128, 512], F32, tag="res")
                        nc.vector.tensor_add(res[:], pout[o][:], bsl[:])
                        nc.sync.dma_start(outT_d[orow:orow+128, :], res[:])

    nc.compile()
    return nc


def _tile_w(wT, ncg):
    """(4096h, ncg*512c) -> (ncg, 32, 128, 512) contiguous slabs."""
    hdim = wT.shape[0]
    return np.ascontiguousarray(
        wT.reshape(hdim // 128, 128, ncg, 512).transpose(2, 0, 1, 3))


def _prep(inputs):
    x = np.asarray(inputs["x"], np.float32)
    mask = np.asarray(inputs["mask"]).astype(bool)
    WQ_w = np.asarray(inputs["WQ_w"], np.float32); WQ_b = np.asarray(inputs["WQ_b"], np.float32)
    WK_w = np.asarray(inputs["WK_w"], np.float32); WK_b = np.asarray(inputs["WK_b"], np.float32)
    WV_w = np.asarray(inputs["WV_w"], np.float32); WV_b = np.asarray(inputs["WV_b"], np.float32)
    W0_w = np.asarray(inputs["W0_w"], np.float32); W0_b = np.asarray(inputs["W0_b"], np.float32)

    causal = bool(np.array_equal(mask, np.triu(np.ones((S, S), bool), k=1)))

    bf = ml_dtypes.bfloat16
    sc = 1.0 / np.sqrt(DQ)
    wq = _tile_w(np.ascontiguousarray((WQ_w * sc).T).astype(bf), 8)
    wk = _tile_w(np.ascontiguousarray(WK_w.T).astype(bf), 2)
    wv = _tile_w(np.ascontiguousarray(WV_w.T).astype(bf), 8)
    w0T = np.ascontiguousarray(W0_w.T).astype(bf)           # (16384, 4096)
    w0 = np.ascontiguousarray(
        w0T.reshape(128, 128, 4, 1024).transpose(2, 0, 1, 3))  # (og, ft, p, 1024)

    # t-permutation within a 128-tile: partition p holds t-offset 8*(p%16) + p//16
    pnew = np.arange(128)
    t_of_p = 8*(pnew % 16) + pnew // 16                     # (128,)

    # V bias (indexed by pnew): V[t, d] bias = WV_b[(t%8)*512 + d]; t%8 = t_of_p%8
    dd_ = np.arange(512)
    bV = WV_b[(t_of_p[:, None] % 8)*512 + dd_[None, :]].astype(bf)

    # diag mask rows permuted: masked iff t_of_p > qq
    qq_ = np.arange(128)
    mask_diag = np.where(t_of_p[:, None] > qq_[None, :], NEG, 0.0).astype(np.float32)

    plane = np.tile(W0_b[:, None], (1, 512)).astype(np.float32)

    maskT_perm = None
    if not causal:
        # maskT[i4, p, c, q'] additive, t = 128*i4 + t_of_p[p], q = 512*c + q'
        madd = np.where(mask.T, NEG, 0.0).astype(np.float32)  # (t, q)
        m4 = madd.reshape(16, 128, 4, 512)
        maskT_perm = np.ascontiguousarray(m4[:, t_of_p, :, :]).astype(bf)

    def fold(v, ntile):
        return np.ascontiguousarray(v.reshape(ntile, 128).T).astype(np.float32)

    bq = fold(WQ_b * sc, 32)
    bk = fold(WK_b, 8)

    in_maps = []
    meta = []
    for b in range(B):
        for r in range(4):
            qtok = np.concatenate(
                [np.arange(512*kk + 128*r, 512*kk + 128*r + 128) for kk in range(4)])
            kvtok = np.arange(512*r, 512*r + 512)
            xq = np.ascontiguousarray(
                x[b][qtok, :].T.reshape(32, 128, 512).transpose(1, 0, 2)).astype(bf)
            xkv = np.ascontiguousarray(
                x[b][kvtok, :].T.reshape(32, 128, 512).transpose(1, 0, 2)).astype(bf)
            m = dict(xq=xq, xkv=xkv, wq=wq, bq=bq, wk=wk, bk=bk, wv=wv,
                     bV=bV, w0=w0, bias_plane=plane, mask_diag=mask_diag)
            if not causal:
                m["maskT"] = maskT_perm
            in_maps.append(m)
            meta.append((b, r))
    return causal, in_maps, meta


def kernel(**inputs):
    causal, in_maps, meta = _prep(inputs)
    if causal not in _CACHE:
        _CACHE[causal] = build(causal)
    nc = _CACHE[causal]
    res = run_bass_kernel_spmd(nc, in_maps, core_ids=list(range(8)))
    out = np.empty((B, S, H), np.float32)
    for i, (b, r) in enumerate(meta):
        outT = res.results[i]["outT"]
        for hloc in range(8):
            hq = 2*r + 8*(hloc // 2) + (hloc % 2)
            out[b, 64*hq:64*hq+64, :] = outT[:, 64*hloc:64*hloc+64].T
    return out



# revision 15
# speedup vs baseline: 1.0762x; 1.0762x over previous
"""Trainium2 Bass kernel for nn_Attention_54520314855575.

GQA attention with raw row-major reshapes (faithful to reference). The raw
reshapes scramble heads/tokens such that each query head's 64 output rows are
disjoint across heads -> shard 8 ways (2 batches x 4 head-groups) with zero
collectives. Per core: 8 query heads (hq%8 in {2r, 2r+1}), K/V heads {2r,2r+1}.

Compute: bf16 matmuls, f32 PSUM/softmax. All projections computed transposed
(channels on partitions) so biases are per-partition; V computed natural and
shuffled on-chip (SBUF->SBUF partition DMAs) into (t, d) layout.

t-axis permutation: within each 128-row t-tile, partition p holds t-offset
8*(p%16) + p//16 (so the V shuffle writes 16 contiguous partitions per
c-chunk). The scores lhsT (KT) is stored in the same order and the diagonal
mask rows are permuted on host, so the contraction stays consistent.

Layout/DMA strategy (v2):
 - all weight streams are >=2MB slab DMAs (descriptor-efficient)
 - ctx stays fully SBUF-resident between phase 2 and phase 3 (128KB/part);
   phase-2 evictions write straight into it (strided DVE), no DRAM scatter
 - QT round-trips through DRAM (8MB contiguous store + 32 contiguous loads)
   to free SBUF for ctx
 - output projection streams w0 as 2MB chunks against SBUF ctx, PSUM
   accumulates the full 128-tile contraction in 2x4-bank halves
"""
import sys, os

for _p in ("/opt/trn_rl_repo",):
    if _p not in sys.path:
        sys.path.append(_p)

import numpy as np
import ml_dtypes

import concourse.bass as bass
import concourse.tile as tile
from concourse import bacc, mybir
from concourse.bass_utils import run_bass_kernel_spmd

BF16 = mybir.dt.bfloat16
F32 = mybir.dt.float32

H = 4096; HQ = 32; HK = 8; HV = 8; DQ = 128; DV = 512; S = 2048; B = 2
NEG = -1.0e30

_CACHE = {}


def build(mode: str):
    """mode: 'causal' | 'nomask' | 'general'"""
    assert mode in ("causal", "nomask", "general")
    causal = mode == "causal"
    nc = bacc.Bacc(None, target_bir_lowering=False, debug=False)

    xq_d = nc.declare_dram_parameter("xq", [128, 32, 512], BF16, isOutput=False)
    xkv_d = nc.declare_dram_parameter("xkv", [128, 32, 512], BF16, isOutput=False)
    wq_d = nc.declare_dram_parameter("wq", [8, 32, 128, 512], BF16, isOutput=False)
    bq_d = nc.declare_dram_parameter("bq", [128, 32], F32, isOutput=False)
    wk_d = nc.declare_dram_parameter("wk", [2, 32, 128, 512], BF16, isOutput=False)
    bk_d = nc.declare_dram_parameter("bk", [128, 8], F32, isOutput=False)
    wv_d = nc.declare_dram_parameter("wv", [8, 32, 128, 512], BF16, isOutput=False)
    bV_d = nc.declare_dram_parameter("bV", [128, 512], BF16, isOutput=False)
    w0_d = nc.declare_dram_parameter("w0", [4, 128, 128, 1024], BF16, isOutput=False)
    b0_d = nc.declare_dram_parameter("b0", [128, 32], F32, isOutput=False)
    mask_diag_d = nc.declare_dram_parameter("mask_diag", [128, 128], F32, isOutput=False)
    if mode == "general":
        maskT_d = nc.declare_dram_parameter("maskT", [16, 128, 4, 512], BF16, isOutput=False)
    outT_d = nc.declare_dram_parameter("outT", [4096, 512], F32, isOutput=True)

    with tile.TileContext(nc) as tc:
        with tc.tile_pool(name="const", bufs=1) as constp, \
             tc.tile_pool(name="dram", bufs=1, space="DRAM") as dramp:

            mask_sb = constp.tile([128, 128], F32)
            nc.sync.dma_start(mask_sb[:], mask_diag_d[:])
            bV_sb = constp.tile([128, 512], BF16)
            nc.sync.dma_start(bV_sb[:], bV_d[:])
            bq_sb = constp.tile([128, 32], F32)
            nc.sync.dma_start(bq_sb[:], bq_d[:])
            bk_sb = constp.tile([128, 8], F32)
            nc.sync.dma_start(bk_sb[:], bk_d[:])
            b0_sb = constp.tile([128, 32], F32)
            nc.sync.dma_start(b0_sb[:], b0_d[:])
            ones_f = constp.tile([128, 1], F32)
            nc.vector.memset(ones_f[:], 1.0)
            ones_r = constp.tile([128, 1], mybir.dt.float32r)
            nc.vector.tensor_copy(ones_r[:], ones_f[:])

            qt_dram = dramp.tile([128, 8, 2048], BF16)  # QT spill [d, hloc, q]

            # KT/Vsh live through phases 1+2 only.
            kvp_cm = tc.tile_pool(name="kv", bufs=1)
            kvp = kvp_cm.__enter__()
            KT = kvp.tile([128, 2, 2048], BF16)       # [d, head j0, tperm]
            Vsh = kvp.tile([128, 2, 16, 512], BF16)   # [pnew, head j0, ttile, d]

            # ---------------- Phase 1: projections ----------------
            # 1a: K and V from xkv; V shuffled into Vsh via SBUF->SBUF DMAs.
            # 1b: Q from xq into QT, then spilled contiguously to qt_dram.
            with tc.tile_pool(name="xres", bufs=1) as xp, \
                 tc.tile_pool(name="qtp", bufs=1) as qtp, \
                 tc.tile_pool(name="wstr", bufs=2) as wp, \
                 tc.tile_pool(name="vtmp", bufs=3) as vtp, \
                 tc.tile_pool(name="pps", bufs=8, space="PSUM") as pps:

                xkv_sb = xp.tile([128, 32, 512], BF16)
                nc.sync.dma_start(xkv_sb[:], xkv_d[:])
                xq_sb = xp.tile([128, 32, 512], BF16)
                nc.sync.dma_start(xq_sb[:], xq_d[:])

                # KT: new within-tile t order: free = 128*i4 + 16*cc + m
                for cg in range(2):
                    acc = [pps.tile([128, 512], F32, tag="pj", name=f"pj{_}") for _ in range(4)]
                    for hh in range(2):
                        wsl = wp.tile([128, 16, 512], BF16, tag="w")
                        nc.sync.dma_start(
                            wsl[:], wk_d[cg, 16*hh:16*hh+16, :, :].rearrange("h p c -> p h c"))
                        for h in range(16):
                            for i in range(4):
                                nc.tensor.matmul(acc[i][:], wsl[:, h, 128*i:128*i+128],
                                                 xkv_sb[:, 16*hh+h, :],
                                                 start=(hh == 0 and h == 0),
                                                 stop=(hh == 1 and h == 15))
                    for i in range(4):
                        ct = 4*cg + i  # == cc
                        # KT[p, hd, 128*i4 + 16*cc + m] <- acc[p, tok=256*hd+16*i4+m]
                        out = KT[:].rearrange("p hd (i4 cc m) -> p hd i4 cc m",
                                              cc=8, m=16)[:, :, :, ct, :]
                        nc.vector.tensor_scalar_add(
                            out, acc[i][:].rearrange("p (hd i4 m) -> p hd i4 m", hd=2, i4=16),
                            bk_sb[:, ct:ct+1])

                # V natural: lhsT = xkv block (h128, tok128), rhs = wv (h128, c512)
                # then shuffle each natural tile into Vsh by 16-partition groups.
                for ccg in range(8):
                    acc = [pps.tile([128, 512], F32, tag="pj", name=f"pj{_}") for _ in range(4)]
                    for hh in range(2):
                        wsl = wp.tile([128, 16, 512], BF16, tag="w")
                        nc.sync.dma_start(
                            wsl[:], wv_d[ccg, 16*hh:16*hh+16, :, :].rearrange("h p c -> p h c"))
                        for h in range(16):
                            for tt in range(4):
                                nc.tensor.matmul(acc[tt][:],
                                                 xkv_sb[:, 16*hh+h, 128*tt:128*tt+128],
                                                 wsl[:, h, :],
                                                 start=(hh == 0 and h == 0),
                                                 stop=(hh == 1 and h == 15))
                    for tt in range(4):
                        vnat = vtp.tile([128, 512], BF16, tag="vn")
                        nc.vector.tensor_copy(vnat[:], acc[tt][:])
                        # Vsh[16cc+m, j0, i4, d] = Vnat[tok=256j0+16i4+m, 512cc+d]
                        # this tile: cc=ccg, j0=tt//2, i4 = 8*(tt%2)+i8, src part 16*i8+m
                        for i8 in range(8):
                            nc.sync.dma_start(
                                Vsh[16*ccg:16*ccg+16, tt//2, 8*(tt % 2)+i8, :],
                                vnat[16*i8:16*i8+16, :])

                # V bias: V[pnew, d] += bV[pnew, d] (host permuted)
                for j0 in range(2):
                    for i4 in range(16):
                        nc.vector.tensor_add(Vsh[:, j0, i4, :], Vsh[:, j0, i4, :], bV_sb[:])

                # QT: lhsT = wq block (h128, c128), rhs = xq (h128, tok512)
                QT = qtp.tile([128, 8, 2048], BF16)   # [d, head hloc, q]
                for cg in range(8):
                    acc = [pps.tile([128, 512], F32, tag="pj", name=f"pj{_}") for _ in range(4)]
                    for hh in range(2):
                        wsl = wp.tile([128, 16, 512], BF16, tag="w")
                        nc.sync.dma_start(
                            wsl[:], wq_d[cg, 16*hh:16*hh+16, :, :].rearrange("h p c -> p h c"))
                        for h in range(16):
                            for i in range(4):
                                nc.tensor.matmul(acc[i][:], wsl[:, h, 128*i:128*i+128],
                                                 xq_sb[:, 16*hh+h, :],
                                                 start=(hh == 0 and h == 0),
                                                 stop=(hh == 1 and h == 15))
                    for i in range(4):
                        ct = 4*cg + i  # == sm
                        # QT[p, hd, q=32u+sm] <- acc[p, tok=128k+64j0+u], hd=2k+j0
                        out = QT[:].rearrange("p hd (u sm) -> p hd u sm", sm=32)[:, :, :, ct]
                        nc.vector.tensor_scalar_add(
                            out, acc[i][:].rearrange("p (hd u) -> p hd u", hd=8),
                            bq_sb[:, ct:ct+1])
                for hloc in range(8):
                    nc.sync.dma_start(qt_dram[:, hloc, :], QT[:, hloc, :])

            # ---------------- Phase 2: attention ----------------
            # ctx stays in SBUF: [dp, sm, dd, s'=(hloc,c,u)], written in place.
            # ctxp spans phases 2+3; kvp (KT/Vsh) is freed between them.
            ctxp_cm = tc.tile_pool(name="ctxp", bufs=1, side="right")
            ctxp = ctxp_cm.__enter__()
            ctx_sb = ctxp.tile([128, 32, 4, 512], BF16)  # [dp, sm, dd, s']

            gen = mode == "general"
            with tc.tile_pool(name="qstr", bufs=2 if gen else 3) as qp, \
                 tc.tile_pool(name="esb", bufs=3 if gen else 4) as ep, \
                 tc.tile_pool(name="nrm", bufs=2) as np_, \
                 tc.tile_pool(name="mstr", bufs=1) as mp, \
                 tc.tile_pool(name="aps", bufs=1, space="PSUM") as aps:

                for c in range(4):
                    nt = 4*c + 4 if causal else 16
                    if mode == "general":
                        msk = mp.tile([128, 16, 512], BF16, tag="mk")
                        nc.sync.dma_start(
                            msk[:], maskT_d[:, :, c, :].rearrange("i p q -> p i q"))
                    for hloc in range(8):
                        k, j0 = hloc // 2, hloc % 2
                        qtile = qp.tile([128, 512], BF16, tag="qt")
                        nc.sync.dma_start(qtile[:], qt_dram[:, hloc, 512*c:512*c+512])
                        Esum = ep.tile([128, 512], mybir.dt.float32r, tag="Esum",
                                       bufs=1 if gen else 2)
                        rs_ps = aps.tile([1, 512], F32, tag="rs")
                        pctx = [aps.tile([128, 512], F32, tag=f"ctx{dd}", name=f"ctx{dd}")
                                for dd in range(4)]
                        for i4 in range(nt):
                            sc_ps = aps.tile([128, 512], F32, tag="sc", bufs=2)
                            nc.tensor.matmul(
                                sc_ps[:], KT[:, j0, 128*i4:128*i4+128],
                                qtile[:], start=True, stop=True)
                            if causal:
                                if 4*c <= i4 < 4*c+4:
                                    q0 = 128*(i4 - 4*c)
                                    nc.vector.tensor_add(sc_ps[:, q0:q0+128],
                                                         sc_ps[:, q0:q0+128], mask_sb[:])
                                m0 = 128*(i4 - 4*c) if i4 > 4*c else 0
                            else:
                                if mode == "general":
                                    nc.vector.tensor_add(sc_ps[:], sc_ps[:], msk[:, i4, :])
                                m0 = 0
                            E = ep.tile([128, 512], BF16, tag="E")
                            if m0 > 0:
                                nc.vector.memset(E[:, 0:m0], 0.0)
                            nc.scalar.activation(E[:, m0:512], sc_ps[:, m0:512],
                                                 mybir.ActivationFunctionType.Exp)
                            if i4 == 0:
                                nc.vector.tensor_copy(Esum[:], E[:])
                            else:
                                nc.vector.tensor_add(Esum[:], Esum[:], E[:])
                            for dd in range(4):
                                nc.tensor.matmul(pctx[dd][:],
                                                 Vsh[:, j0, i4, 128*dd:128*dd+128],
                                                 E[:],
                                                 start=(i4 == 0), stop=(i4 == nt-1))
                        nc.tensor.matmul(rs_ps[:], ones_r[:], Esum[:],
                                         start=True, stop=True)
                        # short normalize chain: recip on (1,512), then broadcast
                        rs_sb = np_.tile([1, 512], F32, tag="rssb")
                        nc.vector.tensor_copy(rs_sb[:], rs_ps[:])
                        rc1_sb = np_.tile([1, 512], F32, tag="rc1")
                        nc.vector.reciprocal(rc1_sb[:], rs_sb[:])
                        rc_sb = np_.tile([128, 512], F32, tag="rc")
                        nc.gpsimd.partition_broadcast(rc_sb[:], rc1_sb[:])
                        perm = "p (u sm) -> p sm u"
                        for dd in range(4):
                            # normalize + evict straight into resident ctx
                            dst = ctx_sb[:, :, dd, 64*hloc+16*c:64*hloc+16*c+16]
                            nc.vector.tensor_mul(
                                dst,
                                pctx[dd][:].rearrange(perm, sm=32),
                                rc_sb[:].rearrange(perm, sm=32))

            kvp_cm.__exit__(None, None, None)

            # ---------------- Phase 3: output projection ----------------
            with tc.tile_pool(name="w0str", bufs=2) as w0p, \
                 tc.tile_pool(name="evo", bufs=1) as evp, \
                 tc.tile_pool(name="wps", bufs=1, space="PSUM") as wps:

                NFT = 8  # w0 chunk: 8 ft tiles = 2MB
                for og in range(4):
                    pout = [wps.tile([128, 512], F32, tag=f"o{o}", name=f"po{o}")
                            for o in range(8)]
                    for ftc in range(128 // NFT):
                        wsl = w0p.tile([128, NFT, 1024], BF16, tag="w0")
                        nc.sync.dma_start(
                            wsl[:], w0_d[og, NFT*ftc:NFT*ftc+NFT, :, :]
                            .rearrange("f p c -> p f c"))
                        for half in range(2):
                            for fl in range(NFT):
                                ft = NFT*ftc + fl
                                sm, dd = ft // 4, ft % 4
                                for o in range(4):
                                    oo = 4*half + o
                                    nc.tensor.matmul(
                                        pout[oo][:], wsl[:, fl, 128*oo:128*oo+128],
                                        ctx_sb[:, sm, dd, :],
                                        start=(ft == 0), stop=(ft == 127))
                    res = evp.tile([128, 8, 512], F32, tag="res")
                    for oo in range(8):
                        nc.vector.tensor_scalar_add(
                            res[:, oo, :], pout[oo][:], b0_sb[:, 8*og+oo:8*og+oo+1])
                    nc.sync.dma_start(
                        outT_d[1024*og:1024*og+1024, :]
                        .rearrange("(o p) s -> p o s", p=128), res[:])

            ctxp_cm.__exit__(None, None, None)

    nc.compile()
    return nc


def _tile_w(wT, ncg):
    """(4096h, ncg*512c) -> (ncg, 32, 128, 512) contiguous slabs."""
    hdim = wT.shape[0]
    return np.ascontiguousarray(
        wT.reshape(hdim // 128, 128, ncg, 512).transpose(2, 0, 1, 3))


def _prep(inputs):
    x = np.asarray(inputs["x"], np.float32)
    mask = np.asarray(inputs["mask"]).astype(bool)
    WQ_w = np.asarray(inputs["WQ_w"], np.float32); WQ_b = np.asarray(inputs["WQ_b"], np.float32)
    WK_w = np.asarray(inputs["WK_w"], np.float32); WK_b = np.asarray(inputs["WK_b"], np.float32)
    WV_w = np.asarray(inputs["WV_w"], np.float32); WV_b = np.asarray(inputs["WV_b"], np.float32)
    W0_w = np.asarray(inputs["W0_w"], np.float32); W0_b = np.asarray(inputs["W0_b"], np.float32)

    if not mask.any():
        mode = "nomask"
    elif np.array_equal(mask, np.triu(np.ones((S, S), bool), k=1)):
        mode = "causal"
    else:
        mode = "general"

    bf = ml_dtypes.bfloat16
    sc = 1.0 / np.sqrt(DQ)
    wq = _tile_w(np.ascontiguousarray((WQ_w * sc).T).astype(bf), 8)
    wk = _tile_w(np.ascontiguousarray(WK_w.T).astype(bf), 2)
    wv = _tile_w(np.ascontiguousarray(WV_w.T).astype(bf), 8)
    w0T = np.ascontiguousarray(W0_w.T).astype(bf)           # (16384, 4096)
    w0 = np.ascontiguousarray(
        w0T.reshape(128, 128, 4, 1024).transpose(2, 0, 1, 3))  # (og, ft, p, 1024)

    # t-permutation within a 128-tile: partition p holds t-offset 8*(p%16) + p//16
    pnew = np.arange(128)
    t_of_p = 8*(pnew % 16) + pnew // 16                     # (128,)

    # V bias (indexed by pnew): V[t, d] bias = WV_b[(t%8)*512 + d]; t%8 = t_of_p%8
    dd_ = np.arange(512)
    bV = WV_b[(t_of_p[:, None] % 8)*512 + dd_[None, :]].astype(bf)

    # diag mask rows permuted: masked iff t_of_p > qq
    qq_ = np.arange(128)
    mask_diag = np.where(t_of_p[:, None] > qq_[None, :], NEG, 0.0).astype(np.float32)

    maskT_perm = None
    if mode == "general":
        # maskT[i4, p, c, q'] additive, t = 128*i4 + t_of_p[p], q = 512*c + q'
        madd = np.where(mask.T, NEG, 0.0).astype(np.float32)  # (t, q)
        m4 = madd.reshape(16, 128, 4, 512)
        maskT_perm = np.ascontiguousarray(m4[:, t_of_p, :, :]).astype(bf)

    def fold(v, ntile):
        return np.ascontiguousarray(v.reshape(ntile, 128).T).astype(np.float32)

    bq = fold(WQ_b * sc, 32)
    bk = fold(WK_b, 8)
    b0 = fold(W0_b, 32)

    in_maps = []
    meta = []
    for b in range(B):
        for r in range(4):
            qtok = np.concatenate(
                [np.arange(512*kk + 128*r, 512*kk + 128*r + 128) for kk in range(4)])
            kvtok = np.arange(512*r, 512*r + 512)
            xq = np.ascontiguousarray(
                x[b][qtok, :].T.reshape(32, 128, 512).transpose(1, 0, 2)).astype(bf)
            xkv = np.ascontiguousarray(
                x[b][kvtok, :].T.reshape(32, 128, 512).transpose(1, 0, 2)).astype(bf)
            m = dict(xq=xq, xkv=xkv, wq=wq, bq=bq, wk=wk, bk=bk, wv=wv,
                     bV=bV, w0=w0, b0=b0, mask_diag=mask_diag)
            if mode == "general":
                m["maskT"] = maskT_perm
            in_maps.append(m)
            meta.append((b, r))
    return mode, in_maps, meta


def kernel(**inputs):
    mode, in_maps, meta = _prep(inputs)
    if mode not in _CACHE:
        _CACHE[mode] = build(mode)
    nc = _CACHE[mode]
    res = run_bass_kernel_spmd(nc, in_maps, core_ids=list(range(8)))
    out = np.empty((B, S, H), np.float32)
    for i, (b, r) in enumerate(meta):
        outT = res.results[i]["outT"]
        for hloc in range(8):
            hq = 2*r + 8*(hloc // 2) + (hloc % 2)
            out[b, 64*hq:64*hq+64, :] = outT[:, 64*hloc:64*hloc+64].T
    return out


# revision 26
# speedup vs baseline: 1.1026x; 1.0245x over previous
"""Trainium2 Bass kernel for nn_Attention_54520314855575.

GQA attention with raw row-major reshapes (faithful to reference). The raw
reshapes scramble heads/tokens such that each query head's 64 output rows are
disjoint across heads -> shard 8 ways (2 batches x 4 head-groups) with zero
collectives. Per core: 8 query heads (hq%8 in {2r, 2r+1}), K/V heads {2r,2r+1}.

Compute: bf16 matmuls, f32 PSUM/softmax. All projections computed transposed
(channels on partitions) so biases are per-partition; V computed natural and
shuffled on-chip (SBUF->SBUF partition DMAs) into (t, d) layout.

t-axis permutation: within each 128-row t-tile, partition p holds t-offset
8*(p%16) + p//16 (so the V shuffle writes 16 contiguous partitions per
c-chunk). The scores lhsT (KT) is stored in the same order and the diagonal
mask rows are permuted on host, so the contraction stays consistent.

Layout/DMA strategy (v2):
 - all weight streams are >=2MB slab DMAs (descriptor-efficient)
 - ctx stays fully SBUF-resident between phase 2 and phase 3 (128KB/part);
   phase-2 evictions write straight into it (strided DVE), no DRAM scatter
 - QT round-trips through DRAM (8MB contiguous store + 32 contiguous loads)
   to free SBUF for ctx
 - output projection streams w0 as 2MB chunks against SBUF ctx, PSUM
   accumulates the full 128-tile contraction in 2x4-bank halves
"""
import sys, os

for _p in ("/opt/trn_rl_repo",):
    if _p not in sys.path:
        sys.path.append(_p)

import numpy as np
import ml_dtypes

import concourse.bass as bass
import concourse.tile as tile
from concourse import bacc, mybir
from concourse.bass_utils import run_bass_kernel_spmd

BF16 = mybir.dt.bfloat16
F32 = mybir.dt.float32

H = 4096; HQ = 32; HK = 8; HV = 8; DQ = 128; DV = 512; S = 2048; B = 2
NEG = -1.0e30

_CACHE = {}


def build(mode: str):
    """mode: 'causal' | 'nomask' | 'general'"""
    assert mode in ("causal", "nomask", "general")
    causal = mode == "causal"
    nc = bacc.Bacc(None, target_bir_lowering=False, debug=False)

    xq_d = nc.declare_dram_parameter("xq", [128, 32, 512], BF16, isOutput=False)
    xkv_d = nc.declare_dram_parameter("xkv", [128, 32, 512], BF16, isOutput=False)
    wq_d = nc.declare_dram_parameter("wq", [8, 32, 128, 512], BF16, isOutput=False)
    bq_d = nc.declare_dram_parameter("bq", [128, 32], F32, isOutput=False)
    wk_d = nc.declare_dram_parameter("wk", [2, 32, 128, 512], BF16, isOutput=False)
    bk_d = nc.declare_dram_parameter("bk", [128, 8], F32, isOutput=False)
    wv_d = nc.declare_dram_parameter("wv", [8, 32, 128, 512], BF16, isOutput=False)
    bV_d = nc.declare_dram_parameter("bV", [128, 512], BF16, isOutput=False)
    w0_d = nc.declare_dram_parameter("w0", [4, 128, 128, 1024], BF16, isOutput=False)
    b0_d = nc.declare_dram_parameter("b0", [128, 32], F32, isOutput=False)
    if mode == "causal":
        mask_diag_d = nc.declare_dram_parameter("mask_diag", [128, 4, 512], F32,
                                                isOutput=False)
    if mode == "general":
        maskT_d = nc.declare_dram_parameter("maskT", [16, 128, 4, 512], BF16, isOutput=False)
    outT_d = nc.declare_dram_parameter("outT", [4096, 512], F32, isOutput=True)

    with tile.TileContext(nc) as tc:
        with tc.tile_pool(name="const", bufs=1) as constp, \
             tc.tile_pool(name="dram", bufs=1, space="DRAM") as dramp:

            bV_sb = constp.tile([128, 512], BF16)
            nc.sync.dma_start(bV_sb[:], bV_d[:])
            bq_sb = constp.tile([128, 32], F32)
            nc.sync.dma_start(bq_sb[:], bq_d[:])
            bk_sb = constp.tile([128, 8], F32)
            nc.sync.dma_start(bk_sb[:], bk_d[:])
            b0_sb = constp.tile([128, 32], F32)
            nc.sync.dma_start(b0_sb[:], b0_d[:])
            if causal:
                mask_sb = constp.tile([128, 4, 512], F32)  # diag tiles, (sm,du) cols
                nc.sync.dma_start(mask_sb[:], mask_diag_d[:])
            ones_f = constp.tile([128, 1], F32)
            nc.vector.memset(ones_f[:], 1.0)
            ones_b = constp.tile([128, 1], BF16)   # rowsum lhsT (rs = 1.T @ E)
            nc.vector.tensor_copy(ones_b[:], ones_f[:])
            onesrow_f = constp.tile([1, 128], F32)
            nc.vector.memset(onesrow_f[:], 1.0)
            onesrow_r = constp.tile([1, 128], mybir.dt.float32r)  # rc bcast lhsT
            nc.vector.tensor_copy(onesrow_r[:], onesrow_f[:])

            qt_dram = dramp.tile([128, 8, 2048], BF16)  # QT spill [d, hloc, q]

            # KT/Vsh live through phases 1+2 only.
            kvp_cm = tc.tile_pool(name="kv", bufs=1)
            kvp = kvp_cm.__enter__()
            KT = kvp.tile([128, 2, 2048], BF16)       # [d, head j0, tperm]
            Vsh = kvp.tile([128, 2, 16, 512], BF16)   # [pnew, head j0, ttile, d]

            # ---------------- Phase 1: projections ----------------
            # 1a: K and V from xkv; V shuffled into Vsh via SBUF->SBUF DMAs.
            # 1b: Q from xq into QT, then spilled contiguously to qt_dram.
            with tc.tile_pool(name="xres", bufs=1) as xp, \
                 tc.tile_pool(name="qtp", bufs=1) as qtp, \
                 tc.tile_pool(name="wstr", bufs=2) as wp, \
                 tc.tile_pool(name="vtmp", bufs=3) as vtp, \
                 tc.tile_pool(name="pps", bufs=8, space="PSUM") as pps:

                xkv_sb = xp.tile([128, 32, 512], BF16)
                nc.sync.dma_start(xkv_sb[:], xkv_d[:])
                xq_sb = xp.tile([128, 32, 512], BF16)  # loaded after V section

                # KT: new within-tile t order: free = 128*i4 + 16*cc + m
                for cg in range(2):
                    acc = [pps.tile([128, 512], F32, tag="pj", name=f"pj{_}") for _ in range(4)]
                    for hh in range(2):
                        wsl = wp.tile([128, 16, 512], BF16, tag="w")
                        nc.sync.dma_start(
                            wsl[:], wk_d[cg, 16*hh:16*hh+16, :, :].rearrange("h p c -> p h c"))
                        for h in range(16):
                            for i in range(4):
                                nc.tensor.matmul(acc[i][:], wsl[:, h, 128*i:128*i+128],
                                                 xkv_sb[:, 16*hh+h, :],
                                                 start=(hh == 0 and h == 0),
                                                 stop=(hh == 1 and h == 15))
                    for i in range(4):
                        ct = 4*cg + i  # == cc
                        # KT[p, hd, 128*i4 + 16*cc + m] <- acc[p, tok=256*hd+16*i4+m]
                        out = KT[:].rearrange("p hd (i4 cc m) -> p hd i4 cc m",
                                              cc=8, m=16)[:, :, :, ct, :]
                        nc.vector.tensor_scalar_add(
                            out, acc[i][:].rearrange("p (hd i4 m) -> p hd i4 m", hd=2, i4=16),
                            bk_sb[:, ct:ct+1])

                # V natural: lhsT = xkv block (h128, tok128), rhs = wv (h128, c512)
                # then shuffle each natural tile into Vsh by 16-partition groups.
                for ccg in range(8):
                    acc = [pps.tile([128, 512], F32, tag="pj", name=f"pj{_}") for _ in range(4)]
                    for hh in range(2):
                        wsl = wp.tile([128, 16, 512], BF16, tag="w")
                        nc.sync.dma_start(
                            wsl[:], wv_d[ccg, 16*hh:16*hh+16, :, :].rearrange("h p c -> p h c"))
                        for h in range(16):
                            for tt in range(4):
                                nc.tensor.matmul(acc[tt][:],
                                                 xkv_sb[:, 16*hh+h, 128*tt:128*tt+128],
                                                 wsl[:, h, :],
                                                 start=(hh == 0 and h == 0),
                                                 stop=(hh == 1 and h == 15))
                    for tt in range(4):
                        vnat = vtp.tile([128, 512], BF16, tag="vn")
                        nc.vector.tensor_copy(vnat[:], acc[tt][:])
                        # Vsh[16cc+m, j0, i4, d] = Vnat[tok=256j0+16i4+m, 512cc+d]
                        # this tile: cc=ccg, j0=tt//2, i4 = 8*(tt%2)+i8, src part 16*i8+m
                        for i8 in range(8):
                            # SWDGE queue: keep these 256 small moves off the
                            # HWDGE FIFO so weight-slab loads aren't blocked.
                            nc.gpsimd.dma_start(
                                Vsh[16*ccg:16*ccg+16, tt//2, 8*(tt % 2)+i8, :],
                                vnat[16*i8:16*i8+16, :])

                # V bias: V[pnew, d] += bV[pnew, d] (host permuted)
                for j0 in range(2):
                    for i4 in range(16):
                        nc.vector.tensor_add(Vsh[:, j0, i4, :], Vsh[:, j0, i4, :], bV_sb[:])

                # QT: lhsT = wq block (h128, c128), rhs = xq (h128, tok512)
                nc.sync.dma_start(xq_sb[:], xq_d[:])
                QT = qtp.tile([128, 8, 2048], BF16)   # [d, head hloc, 512c+16sm+du]
                for cg in range(8):
                    acc = [pps.tile([128, 512], F32, tag="pj", name=f"pj{_}") for _ in range(4)]
                    for hh in range(2):
                        wsl = wp.tile([128, 16, 512], BF16, tag="w")
                        nc.sync.dma_start(
                            wsl[:], wq_d[cg, 16*hh:16*hh+16, :, :].rearrange("h p c -> p h c"))
                        for h in range(16):
                            for i in range(4):
                                nc.tensor.matmul(acc[i][:], wsl[:, h, 128*i:128*i+128],
                                                 xq_sb[:, 16*hh+h, :],
                                                 start=(hh == 0 and h == 0),
                                                 stop=(hh == 1 and h == 15))
                    for i in range(4):
                        ct = 4*cg + i  # == sm
                        # q = 32u+sm stored at col 512c+16sm+du (u = 16c+du):
                        # QT[p, hd, c, ct, du] <- acc[p, tok=64hd+16c+du]
                        out = QT[:].rearrange("p hd (c sm du) -> p hd c sm du",
                                              sm=32, du=16)[:, :, :, ct, :]
                        nc.vector.tensor_scalar_add(
                            out, acc[i][:].rearrange("p (hd c du) -> p hd c du",
                                                     hd=8, c=4),
                            bq_sb[:, ct:ct+1])
                for hloc in range(8):
                    nc.sync.dma_start(qt_dram[:, hloc, :], QT[:, hloc, :])

            # ---------------- Phase 2: attention ----------------
            # ctx stays in SBUF: [dp, sm, dd, s'=(hloc,c,u)], written in place.
            # ctxp spans phases 2+3; kvp (KT/Vsh) is freed between them.
            ctxp_cm = tc.tile_pool(name="ctxp", bufs=1, side="right")
            ctxp = ctxp_cm.__enter__()
            ctx_sb = ctxp.tile([128, 32, 4, 512], BF16)  # [dp, sm, dd, s']

            gen = mode == "general"
            with tc.tile_pool(name="qstr", bufs=2 if gen else 3) as qp, \
                 tc.tile_pool(name="esb", bufs=3 if gen else 4) as ep, \
                 tc.tile_pool(name="nrm", bufs=2) as np_, \
                 tc.tile_pool(name="mstr", bufs=1) as mp, \
                 tc.tile_pool(name="aps", bufs=1, space="PSUM") as aps:

                for c in range(4):
                    nt = 4*c + 4 if causal else 16
                    if mode == "general":
                        msk = mp.tile([128, 16, 512], BF16, tag="mk")
                        nc.sync.dma_start(
                            msk[:], maskT_d[:, :, c, :].rearrange("i p q -> p i q"))
                    for hloc in range(8):
                        k, j0 = hloc // 2, hloc % 2
                        qtile = qp.tile([128, 512], BF16, tag="qt")
                        nc.sync.dma_start(qtile[:], qt_dram[:, hloc, 512*c:512*c+512])
                        rs_ps = aps.tile([1, 512], F32, tag="rs")
                        pctx = [aps.tile([128, 512], F32, tag=f"ctx{dd}", name=f"ctx{dd}")
                                for dd in range(4)]

                        def score(i4):
                            # scores for one 128-t tile + additive mask
                            sc_ps = aps.tile([128, 512], F32, tag="sc", bufs=2)
                            nc.tensor.matmul(
                                sc_ps[:], KT[:, j0, 128*i4:128*i4+128],
                                qtile[:], start=True, stop=True)
                            if causal and 4*c <= i4:
                                nc.vector.tensor_add(sc_ps[:], sc_ps[:],
                                                     mask_sb[:, i4-4*c, :])
                            elif mode == "general":
                                nc.vector.tensor_add(sc_ps[:], sc_ps[:], msk[:, i4, :])
                            return sc_ps

                        # software-pipelined: score(i4+1) is emitted ahead of
                        # ctx(i4) so the PE never waits on exp(i4).
                        sc_cur = score(0)
                        for i4 in range(nt):
                            sc_nxt = score(i4+1) if i4+1 < nt else None
                            E = ep.tile([128, 512], BF16, tag="E")
                            nc.scalar.activation(E[:], sc_cur[:],
                                                 mybir.ActivationFunctionType.Exp)
                            nc.tensor.matmul(rs_ps[:], ones_b[:], E[:],
                                             start=(i4 == 0), stop=(i4 == nt-1))
                            for dd in range(4):
                                nc.tensor.matmul(pctx[dd][:],
                                                 Vsh[:, j0, i4, 128*dd:128*dd+128],
                                                 E[:],
                                                 start=(i4 == 0), stop=(i4 == nt-1))
                            sc_cur = sc_nxt
                        # normalize: recip of rowsums, rank-1 bcast via PE
                        rs_sb = np_.tile([1, 512], F32, tag="rssb")
                        nc.vector.tensor_copy(rs_sb[:], rs_ps[:])
                        rc1_sb = np_.tile([1, 512], mybir.dt.float32r, tag="rc1")
                        with nc.allow_low_precision(reason="f32r == fp32 bits"):
                            nc.vector.reciprocal(rc1_sb[:], rs_sb[:])
                        rc_ps = aps.tile([128, 512], F32, tag="rc")
                        nc.tensor.matmul(rc_ps[:], onesrow_r[:], rc1_sb[:],
                                         start=True, stop=True)
                        rc_sb = np_.tile([128, 512], F32, tag="rcsb")
                        nc.vector.tensor_copy(rc_sb[:], rc_ps[:])
                        perm = "p (sm du) -> p sm du"
                        for dd in range(4):
                            # normalize + evict straight into resident ctx;
                            # sources contiguous, dst in 32B runs
                            dst = ctx_sb[:, :, dd, 64*hloc+16*c:64*hloc+16*c+16]
                            nc.vector.tensor_mul(
                                dst,
                                pctx[dd][:].rearrange(perm, sm=32),
                                rc_sb[:].rearrange(perm, sm=32))

            kvp_cm.__exit__(None, None, None)

            # ---------------- Phase 3: output projection ----------------
            with tc.tile_pool(name="w0str", bufs=2) as w0p, \
                 tc.tile_pool(name="evo", bufs=1) as evp, \
                 tc.tile_pool(name="wps", bufs=1, space="PSUM") as wps:

                NFT = 8  # w0 chunk: 8 ft tiles = 2MB
                for og in range(4):
                    pout = [wps.tile([128, 512], F32, tag=f"o{o}", name=f"po{o}")
                            for o in range(8)]
                    for ftc in range(128 // NFT):
                        wsl = w0p.tile([128, NFT, 1024], BF16, tag="w0")
                        nc.sync.dma_start(
                            wsl[:], w0_d[og, NFT*ftc:NFT*ftc+NFT, :, :]
                            .rearrange("f p c -> p f c"))
                        for half in range(2):
                            for fl in range(NFT):
                                ft = NFT*ftc + fl
                                sm, dd = ft // 4, ft % 4
                                for o in range(4):
                                    oo = 4*half + o
                                    nc.tensor.matmul(
                                        pout[oo][:], wsl[:, fl, 128*oo:128*oo+128],
                                        ctx_sb[:, sm, dd, :],
                                        start=(ft == 0), stop=(ft == 127))
                    res = evp.tile([128, 8, 512], F32, tag="res")
                    for oo in range(8):
                        nc.vector.tensor_scalar_add(
                            res[:, oo, :], pout[oo][:], b0_sb[:, 8*og+oo:8*og+oo+1])
                    nc.sync.dma_start(
                        outT_d[1024*og:1024*og+1024, :]
                        .rearrange("(o p) s -> p o s", p=128), res[:])

            ctxp_cm.__exit__(None, None, None)

    nc.compile()
    return nc


def _tile_w(wT, ncg):
    """(4096h, ncg*512c) -> (ncg, 32, 128, 512) contiguous slabs."""
    hdim = wT.shape[0]
    return np.ascontiguousarray(
        wT.reshape(hdim // 128, 128, ncg, 512).transpose(2, 0, 1, 3))


def _prep(inputs):
    x = np.asarray(inputs["x"], np.float32)
    mask = np.asarray(inputs["mask"]).astype(bool)
    WQ_w = np.asarray(inputs["WQ_w"], np.float32); WQ_b = np.asarray(inputs["WQ_b"], np.float32)
    WK_w = np.asarray(inputs["WK_w"], np.float32); WK_b = np.asarray(inputs["WK_b"], np.float32)
    WV_w = np.asarray(inputs["WV_w"], np.float32); WV_b = np.asarray(inputs["WV_b"], np.float32)
    W0_w = np.asarray(inputs["W0_w"], np.float32); W0_b = np.asarray(inputs["W0_b"], np.float32)

    if not mask.any():
        mode = "nomask"
    elif np.array_equal(mask, np.triu(np.ones((S, S), bool), k=1)):
        mode = "causal"
    else:
        mode = "general"

    bf = ml_dtypes.bfloat16
    sc = 1.0 / np.sqrt(DQ)
    wq = _tile_w(np.ascontiguousarray((WQ_w * sc).T).astype(bf), 8)
    wk = _tile_w(np.ascontiguousarray(WK_w.T).astype(bf), 2)
    wv = _tile_w(np.ascontiguousarray(WV_w.T).astype(bf), 8)
    w0T = np.ascontiguousarray(W0_w.T).astype(bf)           # (16384, 4096)
    w0 = np.ascontiguousarray(
        w0T.reshape(128, 128, 4, 1024).transpose(2, 0, 1, 3))  # (og, ft, p, 1024)

    # t-permutation within a 128-tile: partition p holds t-offset 8*(p%16) + p//16
    pnew = np.arange(128)
    t_of_p = 8*(pnew % 16) + pnew // 16                     # (128,)

    # V bias (indexed by pnew): V[t, d] bias = WV_b[(t%8)*512 + d]; t%8 = t_of_p%8
    dd_ = np.arange(512)
    bV = WV_b[(t_of_p[:, None] % 8)*512 + dd_[None, :]].astype(bf)

    # Phase-2 q columns are stored as col = 512c + 16sm + du <-> q = 512c+32du+sm.
    # diag tiles (i4 = 4c+j): masked iff 128j + t_of_p > 32du + sm
    mask_diag = None
    if mode == "causal":
        j_ = np.arange(4)[None, :, None]
        sm_ = (np.arange(512) // 16)[None, None, :]
        du_ = (np.arange(512) % 16)[None, None, :]
        mask_diag = np.where(
            128*j_ + t_of_p[:, None, None] > 32*du_ + sm_, NEG, 0.0
        ).astype(np.float32)                                 # (128, 4, 512)

    maskT_perm = None
    if mode == "general":
        # maskT[i4, p, c, 16sm+du] additive, t = 128i4 + t_of_p[p], q = 512c+32du+sm
        madd = np.where(mask.T, NEG, 0.0).astype(np.float32)  # (t, q)
        m5 = madd.reshape(16, 128, 4, 16, 32)                 # (i4, tp, c, du, sm)
        m5 = m5[:, t_of_p, :, :, :].transpose(0, 1, 2, 4, 3)  # (i4, p, c, sm, du)
        maskT_perm = np.ascontiguousarray(m5.reshape(16, 128, 4, 512)).astype(bf)

    def fold(v, ntile):
        return np.ascontiguousarray(v.reshape(ntile, 128).T).astype(np.float32)

    bq = fold(WQ_b * sc, 32)
    bk = fold(WK_b, 8)
    b0 = fold(W0_b, 32)

    in_maps = []
    meta = []
    for b in range(B):
        for r in range(4):
            qtok = np.concatenate(
                [np.arange(512*kk + 128*r, 512*kk + 128*r + 128) for kk in range(4)])
            kvtok = np.arange(512*r, 512*r + 512)
            xq = np.ascontiguousarray(
                x[b][qtok, :].T.reshape(32, 128, 512).transpose(1, 0, 2)).astype(bf)
            xkv = np.ascontiguousarray(
                x[b][kvtok, :].T.reshape(32, 128, 512).transpose(1, 0, 2)).astype(bf)
            m = dict(xq=xq, xkv=xkv, wq=wq, bq=bq, wk=wk, bk=bk, wv=wv,
                     bV=bV, w0=w0, b0=b0)
            if mode == "causal":
                m["mask_diag"] = mask_diag
            if mode == "general":
                m["maskT"] = maskT_perm
            in_maps.append(m)
            meta.append((b, r))
    return mode, in_maps, meta


def kernel(**inputs):
    mode, in_maps, meta = _prep(inputs)
    if mode not in _CACHE:
        _CACHE[mode] = build(mode)
    nc = _CACHE[mode]
    res = run_bass_kernel_spmd(nc, in_maps, core_ids=list(range(8)))
    out = np.empty((B, S, H), np.float32)
    for i, (b, r) in enumerate(meta):
        outT = res.results[i]["outT"]
        for hloc in range(8):
            hq = 2*r + 8*(hloc // 2) + (hloc % 2)
            out[b, 64*hq:64*hq+64, :] = outT[:, 64*hloc:64*hloc+64].T
    return out
